# revision 2
# baseline (speedup 1.0000x reference)
"""Bahdanau encoder-decoder LSTM on 8 Trainium2 NeuronCores — v2.

Strategy: data-parallel over batch (B=32 -> 4 rows per core), weights
replicated, zero collectives. v2 restructures the per-step program to
shorten the serial cross-engine chain:

- One fused gate activation per LSTM step: the g-gate weight/bias rows
  are pre-scaled by 2 on the host so tanh(0.5*z) yields all four gate
  nonlinearities in a single ACT instruction (sigmoid via tanh).
- Cell state kept doubled (C = 2c); the update needs only 3 fused
  scalar_tensor_tensor ops (split across DVE and the Pool engine) and
  tanh(c) = tanh(0.5*C) folds the halving into the ACT scale.
- h^T produced directly by two tiny scale-matmuls (lhsT = gate slice,
  rhs = 0.5*I / I) plus one fused stt, writing straight into the
  persistent transposed h history (houtT). No per-step output DMA; one
  bulk DMA of the bf16 history at the end.
- dscore^T computed directly in transposed form with stationary
  Wdec^T tiles (no PSUM->SBUF->transpose round trip).
- Elementwise copies/scatters moved to the otherwise-idle Pool engine.
"""
import numpy as np
import ml_dtypes

import concourse.bass as bass
import concourse.tile as tile_mod
from concourse import mybir
from concourse.bass_utils import run_bass_kernel_spmd
from concourse.tile import TileContext
from concourse.vector_clock import ScopedClock

F32 = mybir.dt.float32
BF16 = mybir.dt.bfloat16
FP8 = mybir.dt.float8e4
DR = mybir.MatmulPerfMode.DoubleRow
AF = mybir.ActivationFunctionType
OP = mybir.AluOpType
bf16 = ml_dtypes.bfloat16
f8 = getattr(ml_dtypes, "float8_e4m3fn", None) or ml_dtypes.float8_e4m3

F, HE, HD, A = 128, 512, 512, 256
B, T = 32, 512
NCORES = 8
BL = B // NCORES  # 4 batch rows per core
G = 4 * HD        # 2048 gate width

# ----------------------------------------------------------------------
# Toolchain workarounds: this walrus build refuses any TPB instruction
# carrying more than one semaphore wait. Hoist extras onto standalone
# EventSemaphore instructions (engine program order keeps semantics).
_TPB_ENGINES = None


def _tpb_engines():
    global _TPB_ENGINES
    if _TPB_ENGINES is None:
        _TPB_ENGINES = {
            mybir.EngineType.PE,
            mybir.EngineType.DVE,
            mybir.EngineType.Activation,
            mybir.EngineType.Pool,
            mybir.EngineType.SP,
        }
    return _TPB_ENGINES


def split_multi_waits(nc, cap=1):
    fn = nc.m.functions[0]
    engines = _tpb_engines()
    for bb in fn.blocks:
        insts = bb.instructions
        out = []
        changed = False
        for inst in insts:
            si = inst.sync_info
            waits = None if si is None else si.on_wait
            if waits is not None and len(waits) > cap and inst.engine in engines:
                extra = list(waits[cap:])
                si.on_wait = list(waits[:cap])
                for j, w in enumerate(extra):
                    ev = mybir.InstEventSemaphore(
                        name=f"{inst.name}-xw{j}", ins=[], outs=[]
                    )
                    ev.engine = inst.engine
                    ev.sync_info = mybir.SyncInfo(on_wait=[w], on_update=[])
                    out.append(ev)
                changed = True
            out.append(inst)
        if changed:
            bb.instructions = out


_patched = False


def patch_tile_drain():
    """Same walrus limitation for the Tile tail drain."""
    global _patched
    if _patched:
        return
    _patched = True

    def _drain(self, tick_clock, wait_clock):
        drain_inst = self.nc.sync.drain()
        wait_clock.add_sem_waits(
            drain_inst.ins, ScopedClock({None: tick_clock.global_clock})
        )
        si = drain_inst.ins.sync_info
        if si is not None and si.on_wait is not None and len(si.on_wait) > 1:
            extra = list(si.on_wait[1:])
            si.on_wait = [si.on_wait[0]]
            for w in extra:
                n2 = self.nc.sync.nop()
                n2.ins.sync_info = mybir.SyncInfo(on_wait=[w], on_update=[])
        self.nc.all_engine_barrier()
        popped = self.nc._tile_sem_poison_stack.pop()
        assert popped is self._sem_poison
        self.nc.clear_and_free_semaphores(list(self.sems.allocated().values()))
        self.nc.all_engine_barrier()

    tile_mod.TileContext._drain_and_barrier = _drain


# ----------------------------------------------------------------------
def build_nc(nt=T, dbg=False):
    """Build the single-core Bass program (SPMD across 8 cores)."""
    patch_tile_drain()
    nc = bass.Bass("TRN2", target_bir_lowering=False, debug=False)
    dbg_outs = {}

    def tap(name, ap_or_tile, shape):
        if not dbg:
            return
        d = nc.dram_tensor("dbg_" + name, list(shape), ap_or_tile.dtype,
                           kind="ExternalOutput")
        nc.sync.dma_start(out=d.ap(), in_=ap_or_tile)
        dbg_outs[name] = d

    # ---- DRAM parameters (host-prepped; bf16 unless noted) ----
    d_xT = nc.dram_tensor("xT", [128, nt * BL], BF16, kind="ExternalInput")
    d_wihT = nc.dram_tensor("wihT", [128, G], BF16, kind="ExternalInput")
    d_whhT = nc.dram_tensor("whhT", [4, 128, G], FP8, kind="ExternalInput")
    d_encb = nc.dram_tensor("encb", [1, G], BF16, kind="ExternalInput")
    d_dhr = nc.dram_tensor("dhr", [4, 128, G], BF16, kind="ExternalInput")
    d_dcr = nc.dram_tensor("dcr", [4, 128, G], BF16, kind="ExternalInput")
    d_decb = nc.dram_tensor("decb", [1, G], BF16, kind="ExternalInput")
    d_wdecT = nc.dram_tensor("wdecT", [8, 128, 128], BF16, kind="ExternalInput")
    d_wencT = nc.dram_tensor("wencT", [4, 128, A], BF16, kind="ExternalInput")
    d_epb = nc.dram_tensor("epb", [2, 128, 1], F32, kind="ExternalInput")
    d_vblk = nc.dram_tensor("vblk", [128, 8 * 16], FP8, kind="ExternalInput")
    d_ones = nc.dram_tensor("ones", [1, BL], BF16, kind="ExternalInput")
    d_id4b = nc.dram_tensor("id4b", [BL, BL], BF16, kind="ExternalInput")
    d_id4f = nc.dram_tensor("id4f", [BL, BL], F32, kind="ExternalInput")
    d_half4f = nc.dram_tensor("half4f", [BL, BL], F32, kind="ExternalInput")
    d_id128 = nc.dram_tensor("id128", [128, 128], BF16, kind="ExternalInput")
    d_hd0T = nc.dram_tensor("hd0T", [128, 4 * BL], BF16, kind="ExternalInput")
    d_cd0x2 = nc.dram_tensor("cd0x2", [BL, HD], F32, kind="ExternalInput")
    d_out = nc.dram_tensor("out", [128, nt * 4 * BL], BF16, kind="ExternalOutput")

    from contextlib import ExitStack
    with TileContext(nc) as tc, ExitStack() as ctx:
        const = ctx.enter_context(tc.tile_pool(name="const", bufs=1))
        state = ctx.enter_context(tc.tile_pool(name="state", bufs=1))
        work = ctx.enter_context(tc.tile_pool(name="work", bufs=1))

        # ---- load constants into SBUF ----
        xT = const.tile([128, nt * BL], BF16)
        nc.sync.dma_start(out=xT, in_=d_xT.ap())
        wihT = const.tile([128, G], BF16)
        nc.sync.dma_start(out=wihT, in_=d_wihT.ap())
        whhT = const.tile([128, 4, G], FP8)
        for k in range(4):
            nc.sync.dma_start(out=whhT[:, k, :], in_=d_whhT.ap()[k])
        encb = const.tile([1, G], BF16)
        nc.sync.dma_start(out=encb, in_=d_encb.ap())
        dhr = const.tile([128, 4, G], BF16)
        for k in range(4):
            nc.sync.dma_start(out=dhr[:, k, :], in_=d_dhr.ap()[k])
        dcr = const.tile([128, 4, G], BF16)
        for k in range(4):
            nc.sync.dma_start(out=dcr[:, k, :], in_=d_dcr.ap()[k])
        decb = const.tile([1, G], BF16)
        nc.sync.dma_start(out=decb, in_=d_decb.ap())
        wdecT = const.tile([128, 8, 128], BF16)
        for k in range(8):
            nc.sync.dma_start(out=wdecT[:, k, :], in_=d_wdecT.ap()[k])
        wencT = const.tile([128, 4, A], BF16)
        for k in range(4):
            nc.sync.dma_start(out=wencT[:, k, :], in_=d_wencT.ap()[k])
        epb = const.tile([128, 2], F32)
        for k in range(2):
            nc.sync.dma_start(out=epb[:, k:k + 1], in_=d_epb.ap()[k])
        vblk = const.tile([128, 8 * 16], FP8)
        nc.sync.dma_start(out=vblk, in_=d_vblk.ap())
        ones = const.tile([1, BL], BF16)
        nc.sync.dma_start(out=ones, in_=d_ones.ap())
        id4b = const.tile([BL, BL], BF16)
        nc.sync.dma_start(out=id4b, in_=d_id4b.ap())
        id4f = const.tile([BL, BL], F32)
        nc.sync.dma_start(out=id4f, in_=d_id4f.ap())
        half4f = const.tile([BL, BL], F32)
        nc.sync.dma_start(out=half4f, in_=d_half4f.ap())
        id128 = const.tile([128, 128], BF16)
        nc.sync.dma_start(out=id128, in_=d_id128.ap())

        # ---- persistent state ----
        # encoder h history, transposed: [e', (ec, t, b)]
        encT = state.tile([128, 4, nt * BL], BF16)
        # fp8 shadow of encT for the DoubleRow Whh matmuls
        encT8 = state.tile([128, 4, nt * BL], FP8)
        # decoder h history, transposed: [e', (t_slot, ec, b)]; slot 0 = init
        houtT = state.tile([128, (nt + 1) * 4 * BL], BF16)
        nc.sync.dma_start(out=houtT[:, 0:4 * BL], in_=d_hd0T.ap())
        C_enc = state.tile([BL, HE], F32)   # = 2*c
        nc.vector.memset(C_enc, 0.0)
        C_dec = state.tile([BL, HD], F32)   # = 2*c
        nc.sync.dma_start(out=C_dec, in_=d_cd0x2.ap())
        enc_projT = state.tile([128, 2, BL, nt], F32)   # [a', (ac, b, t)]
        enc_tb = state.tile([128, BL, 4, HE], FP8)      # [t', (b, tc, e)]
        align = state.tile([128, 2, BL, nt], FP8)       # [a', (ac, b, t)]
        # 16 fp8 block-diag chunks of width 16 (DoubleRow k-pair layout);
        # block (b, tc) holds w^T col b at flat col 65*b + 16*tc.
        wTblk = state.tile([128, 16 * 16 + 16], FP8)
        nc.vector.memset(wTblk, 0.0)
        ctxT = state.tile([128, 4 * BL], BF16)

        # ---- per-step work tiles ----
        Gt = work.tile([BL, G], F32)
        u_t = work.tile([BL, HE], F32)
        v_t = work.tile([BL, HE], F32)
        th = work.tile([BL, HE], F32)
        oT_s = work.tile([128, 4 * BL], F32)
        dsT_sb = work.tile([128, 2 * BL], F32)
        w_u = work.tile([BL, T], F32)
        denom = work.tile([BL, 1], F32)
        rcp = work.tile([BL, 1], F32)
        diag_s = work.tile([BL, BL], F32)
        ctx_s = work.tile([BL, HE], BF16)

        def lstm_tail(z_ps, C_st, psm, toT_sl, thT_sl, out_ap):
            """Fused gate tail. z -> gates -> C update -> h^T into out_ap.

            toT_sl/thT_sl: [128, 4*BL] column ranges in the small psum tile.
            out_ap: [128, 4*BL]-shaped AP (possibly strided) for h^T.
            """
            # gates via tanh(0.5*z) (g-gate weights pre-doubled); i|f|g
            # first so the DVE chain starts sooner, o hides under it
            nc.scalar.activation(out=Gt[:, 0:3 * HE], in_=z_ps[:, 0:3 * HE],
                                 func=AF.Tanh, bias=0.0, scale=0.5)
            nc.scalar.activation(out=Gt[:, 3 * HE:], in_=z_ps[:, 3 * HE:],
                                 func=AF.Tanh, bias=0.0, scale=0.5)
            # u = (ti+1)*tg = 2*sig(i)*tanh(g)      (DVE)
            nc.vector.scalar_tensor_tensor(
                out=u_t, in0=Gt[:, 0:HE], scalar=1.0, in1=Gt[:, 2 * HE:3 * HE],
                op0=OP.add, op1=OP.mult)
            # v = (tf+1)*C = 4*sig(f)*c             (DVE)
            nc.vector.scalar_tensor_tensor(
                out=v_t, in0=Gt[:, HE:2 * HE], scalar=1.0, in1=C_st,
                op0=OP.add, op1=OP.mult)
            # C' = 0.5*v + u = 2*c_new              (DVE)
            nc.vector.scalar_tensor_tensor(
                out=C_st, in0=v_t, scalar=0.5, in1=u_t,
                op0=OP.mult, op1=OP.add)
            # th = tanh(c_new)
            nc.scalar.activation(out=th, in_=C_st, func=AF.Tanh,
                                 bias=0.0, scale=0.5)
            # toT (scaled 0.5) and thT via tiny matmuls; all to-matmuls
            # first so they run on PE while ACT computes th
            for ec in range(4):
                nc.tensor.matmul(
                    psm[:, toT_sl + ec * BL: toT_sl + (ec + 1) * BL],
                    Gt[:, 3 * HE + ec * 128: 3 * HE + (ec + 1) * 128],
                    half4f, start=True, stop=True)
            for ec in range(4):
                nc.tensor.matmul(
                    psm[:, thT_sl + ec * BL: thT_sl + (ec + 1) * BL],
                    th[:, ec * 128:(ec + 1) * 128],
                    id4f, start=True, stop=True)
            # sig(o)^T = 0.5*toT + 0.5 -> SBUF (only one PSUM input allowed
            # per DVE op, so stage this part; it is off the critical path)
            nc.vector.tensor_scalar(
                out=oT_s, in0=psm[:, toT_sl: toT_sl + 4 * BL],
                scalar1=0.5, scalar2=None, op0=OP.add)
            # h^T = sig(o)^T * thT
            nc.vector.tensor_tensor(
                out=out_ap, in0=oT_s,
                in1=psm[:, thT_sl: thT_sl + 4 * BL], op=OP.mult)

        # ================= ENCODER =================
        with tc.tile_pool(name="eps", bufs=1, space="PSUM") as eps, \
             tc.tile_pool(name="esm", bufs=1, space="PSUM") as esm:
            psm_e = esm.tile([128, 8 * BL], F32)
            for t in range(nt):
                z_ps = eps.tile([BL, G], F32, tag="z")
                for ng in range(4):
                    sl = slice(ng * HE, (ng + 1) * HE)
                    nc.tensor.matmul(z_ps[:, sl], xT[:, BL * t:BL * (t + 1)],
                                     wihT[:, sl], start=True, stop=False)
                    nc.tensor.matmul(z_ps[:, sl], ones, encb[:, sl],
                                     start=False, stop=(t == 0))
                    if t > 0:
                        for kcp in range(2):
                            nc.tensor.matmul(
                                z_ps[:, sl],
                                encT8[:, 2 * kcp:2 * kcp + 2,
                                      BL * (t - 1): BL * t],
                                whhT[:, 2 * kcp:2 * kcp + 2, sl],
                                start=False, stop=(kcp == 1), perf_mode=DR)
                lstm_tail(z_ps, C_enc, psm_e, 0, 4 * BL,
                          encT[:, :, BL * t: BL * (t + 1)])
                # fp8 shadow for the next step's DoubleRow matmuls (DVE:
                # same queue as the h^T write, no extra cross-engine hop)
                nc.vector.tensor_copy(encT8[:, :, BL * t: BL * (t + 1)],
                                      encT[:, :, BL * t: BL * (t + 1)])
                if t == 0 and dbg:
                    zc = work.tile([BL, G], F32, tag="dbg_z")
                    nc.vector.tensor_copy(zc, z_ps)
                    tap("enc_z0", zc, [BL, G])
            tap("encT", encT, [128, 4 * nt * BL])

        # ================= PHASE 2: enc_projT and enc_tb =================
        with tc.tile_pool(name="p2a", bufs=4, space="PSUM") as p2a, \
             tc.tile_pool(name="p2b", bufs=4, space="PSUM") as p2b:
            for ac in range(2):
                for b in range(BL):
                    pp = p2a.tile([128, nt], F32, tag="pp")
                    for ec in range(4):
                        rhs = encT[:, ec, b: BL * (nt - 1) + b + 1: BL]
                        nc.tensor.matmul(
                            pp, wencT[:, ec, ac * 128:(ac + 1) * 128],
                            rhs, start=(ec == 0), stop=(ec == 3))
                    nc.scalar.activation(
                        out=enc_projT[:, ac, b, :], in_=pp,
                        func=AF.Identity, bias=epb[:, ac:ac + 1], scale=1.0)
            for b in range(BL):
                for tc_i in range(nt // 128):
                    for ec in range(4):
                        src = encT[:, ec, tc_i * 512 + b: tc_i * 512 + b
                                   + BL * 127 + 1: BL]
                        pt2 = p2b.tile([128, 128], BF16, tag="pt2")
                        nc.tensor.transpose(pt2, src, id128)
                        nc.vector.tensor_copy(
                            enc_tb[:, b, tc_i, ec * 128:(ec + 1) * 128], pt2)

        tap("enc_projT", enc_projT, [128, 2 * nt * BL])
        tap("enc_tb", enc_tb, [128, 16 * HE])

        # ================= DECODER =================
        with tc.tile_pool(name="dz", bufs=1, space="PSUM") as dz, \
             tc.tile_pool(name="datt", bufs=1, space="PSUM") as datt, \
             tc.tile_pool(name="dsm", bufs=1, space="PSUM") as dsm:
            # small psum: [0:8]=dsT, [8:24]=toT, [24:40]=thT,
            # [40:56]=wT, [56:72]=ctxT
            psm = dsm.tile([128, 72], F32)
            for t in range(nt):
                hsl = houtT[:, t * 4 * BL: (t + 1) * 4 * BL]
                # --- dsT: stationary Wdec^T tiles ---
                for ac in range(2):
                    for kc in range(4):
                        nc.tensor.matmul(
                            psm[:, ac * BL:(ac + 1) * BL],
                            wdecT[:, kc * 2 + ac, :],
                            hsl[:, kc * BL:(kc + 1) * BL],
                            start=(kc == 0), stop=(kc == 3))
                # PSUM->SBUF bias staging on ACT itself: same queue as the
                # align activations, so no extra cross-engine hop
                nc.scalar.copy(dsT_sb, psm[:, 0:2 * BL])
                # --- z h-pass + bias (fills the PE while ACT runs align) ---
                z_ps = dz.tile([BL, G], F32, tag="zd")
                for ng in range(4):
                    sl = slice(ng * HD, (ng + 1) * HD)
                    nc.tensor.matmul(z_ps[:, sl], ones, decb[:, sl],
                                     start=True, stop=False)
                    for kc in range(4):
                        nc.tensor.matmul(
                            z_ps[:, sl], hsl[:, kc * BL:(kc + 1) * BL],
                            dhr[:, kc, sl], start=False, stop=False)
                # --- attention tanh ---
                for ac in range(2):
                    for b in range(BL):
                        nc.scalar.activation(
                            out=align[:, ac, b, :],
                            in_=enc_projT[:, ac, b, :],
                            func=AF.Tanh,
                            bias=dsT_sb[:, ac * BL + b: ac * BL + b + 1],
                            scale=1.0)
                # --- V reduction (block-diag, DoubleRow over ac pairs) ---
                lg_ps = datt.tile([BL, T], F32, tag="lg")
                for b in range(BL):
                    lhsT = vblk[:, b * 32: b * 32 + 32].rearrange(
                        "p (a w) -> p a w", a=2)[:, :, 0:BL]
                    nc.tensor.matmul(
                        lg_ps[:, :nt], lhsT, align[:, 0:2, b, :],
                        start=(b == 0), stop=(b == 3), perf_mode=DR)
                # --- softmax (V_b cancels; logits bounded) ---
                nc.scalar.activation(out=w_u[:, :nt], in_=lg_ps[:, :nt],
                                     func=AF.Exp, bias=0.0, scale=1.0,
                                     accum_out=denom)
                nc.vector.reciprocal(rcp, denom)
                # diag(1/denom): folds softmax normalization into the wT
                # transpose-matmuls, removing the ctx normalize from the chain
                nc.vector.tensor_scalar(out=diag_s, in0=id4f, scalar1=rcp,
                                        scalar2=None, op0=OP.mult)
                # --- normalized wT via scale-matmuls into fp8 block-diag ---
                # block (b, tc) base = (b*4+tc)*16, col b -> flat 65*b + 16*tc
                for tc_i in range(4):
                    wtp = psm[:, 40 + tc_i * BL: 40 + (tc_i + 1) * BL]
                    nc.tensor.matmul(wtp, w_u[:, tc_i * 128:(tc_i + 1) * 128],
                                     diag_s, start=True, stop=True)
                    dst = wTblk[:, 16 * tc_i: 16 * tc_i + 65 * (BL - 1) + 1: 65]
                    nc.vector.tensor_copy(dst, wtp)
                # --- ctx (unnormalized, DoubleRow over tc pairs) ---
                ctx_ps = datt.tile([BL, HE], F32, tag="ctx")
                q = 0
                for b in range(BL):
                    for tcp in range(2):
                        lhsT = wTblk[:, (b * 4 + 2 * tcp) * 16:
                                     (b * 4 + 2 * tcp) * 16 + 32].rearrange(
                            "p (a w) -> p a w", a=2)[:, :, 0:BL]
                        nc.tensor.matmul(
                            ctx_ps, lhsT,
                            enc_tb[:, b, 2 * tcp:2 * tcp + 2, :],
                            start=(q == 0), stop=(q == 7), perf_mode=DR)
                        q += 1
                # --- ctx is already normalized; plain cast to bf16 ---
                nc.vector.tensor_copy(ctx_s, ctx_ps)
                # --- ctxT via scale-matmuls ---
                for ec in range(4):
                    ctp = psm[:, 56 + ec * BL: 56 + (ec + 1) * BL]
                    nc.tensor.matmul(ctp, ctx_s[:, ec * 128:(ec + 1) * 128],
                                     id4b, start=True, stop=True)
                    nc.vector.tensor_copy(ctxT[:, ec * BL:(ec + 1) * BL], ctp)
                # --- cell ctx-pass (accumulate into z) ---
                for ng in range(4):
                    sl = slice(ng * HD, (ng + 1) * HD)
                    for kc in range(4):
                        nc.tensor.matmul(
                            z_ps[:, sl], ctxT[:, kc * BL:(kc + 1) * BL],
                            dcr[:, kc, sl], start=False, stop=(kc == 3))
                # --- gates + state + h^T into houtT slot t+1 ---
                lstm_tail(z_ps, C_dec, psm, 8, 24,
                          houtT[:, (t + 1) * 4 * BL: (t + 2) * 4 * BL])
                if t == 0 and dbg:
                    tap("d_dsT0", dsT_sb, [128, 2 * BL])
                    tap("d_wu0", w_u, [BL, T])
                    tap("d_denom0", denom, [BL, 1])
                    tap("d_ctxs0", ctx_s, [BL, HE])
        # one bulk DMA of the h history (excluding the init slot)
        nc.sync.dma_start(out=d_out.ap(), in_=houtT[:, 4 * BL:])

    split_multi_waits(nc)
    if dbg:
        nc._dbg_outs = dbg_outs
    return nc


# ----------------------------------------------------------------------
def _sig(x):
    return 1.0 / (1.0 + np.exp(-x))


def prepare_inputs(inputs, nt=T):
    """Host-side weight/layout prep. Returns per-core input map fn."""
    f32 = np.float32
    enc_Wih = np.asarray(inputs["enc_Wih"], f32)
    enc_Whh = np.asarray(inputs["enc_Whh"], f32)
    enc_bias = np.asarray(inputs["enc_bih"], f32) + np.asarray(inputs["enc_bhh"], f32)
    Wenc_w = np.asarray(inputs["Wenc_w"], f32)
    Wenc_b = np.asarray(inputs["Wenc_b"], f32)
    Wdec_w = np.asarray(inputs["Wdec_w"], f32)
    Wdec_b = np.asarray(inputs["Wdec_b"], f32)
    V_w = np.asarray(inputs["V_w"], f32)
    attn_bias = np.asarray(inputs["attn_bias"], f32)
    dec_Wih = np.asarray(inputs["dec_Wih"], f32)
    dec_Whh = np.asarray(inputs["dec_Whh"], f32)
    dec_bias = np.asarray(inputs["dec_bih"], f32) + np.asarray(inputs["dec_bhh"], f32)

    def g2(w):
        """Double the g-gate block (rows 2H:3H of the 4H gate dim)."""
        w = w.copy()
        hq = w.shape[0] // 4
        w[2 * hq:3 * hq] *= 2.0
        return w

    sh = {}
    sh["wihT"] = np.ascontiguousarray(g2(enc_Wih).T).astype(bf16)
    # Whh.T is [HE, 4HE]; k-chunks split HE  (fp8 for DoubleRow)
    sh["whhT"] = np.ascontiguousarray(
        g2(enc_Whh).T.reshape(4, 128, 4 * HE)).astype(f8)
    sh["encb"] = g2(enc_bias.reshape(-1, 1))[:, 0].reshape(1, -1).astype(bf16)
    dec_h_w = g2(dec_Wih[:, HD:] + dec_Whh)          # [2048, 512]
    sh["dhr"] = np.ascontiguousarray(
        dec_h_w.T.reshape(4, 128, G)).astype(bf16)
    sh["dcr"] = np.ascontiguousarray(
        g2(dec_Wih[:, :HD]).T.reshape(4, 128, G)).astype(bf16)
    sh["decb"] = g2(dec_bias.reshape(-1, 1))[:, 0].reshape(1, -1).astype(bf16)
    # Wdec^T tiles [kc, ac] -> [8, 128, 128], index kc*2+ac
    wdT = Wdec_w.T  # [HD, A]
    wt = np.zeros((8, 128, 128), f32)
    for kc in range(4):
        for ac in range(2):
            wt[kc * 2 + ac] = wdT[kc * 128:(kc + 1) * 128,
                                  ac * 128:(ac + 1) * 128]
    sh["wdecT"] = wt.astype(bf16)
    sh["wencT"] = np.ascontiguousarray(
        Wenc_w.T.reshape(4, 128, A)).astype(bf16)
    sh["epb"] = np.ascontiguousarray(
        (Wenc_b + attn_bias + Wdec_b).reshape(2, 128, 1)).astype(f32)
    # V block-diag for DoubleRow: block (b, ac) at (b*2+ac)*16, col b
    vb = np.zeros((128, 8 * 16), f32)
    for b in range(BL):
        for ac in range(2):
            vb[:, (b * 2 + ac) * 16 + b] = V_w[0, ac * 128:(ac + 1) * 128]
    sh["vblk"] = vb.astype(f8)
    sh["ones"] = np.ones((1, BL), f32).astype(bf16)
    sh["id4b"] = np.eye(BL, dtype=f32).astype(bf16)
    sh["id4f"] = np.eye(BL, dtype=f32)
    sh["half4f"] = (0.5 * np.eye(BL)).astype(f32)
    sh["id128"] = np.eye(128, dtype=f32).astype(bf16)
    # decoder init state (z0 from biases only; original biases, no g2)
    i0, f0, g0, o0 = np.split(dec_bias, 4)
    cd0 = _sig(i0) * np.tanh(g0)
    hd0 = _sig(o0) * np.tanh(cd0)
    hd0T = np.zeros((128, 4 * BL), f32)
    for ec in range(4):
        for b in range(BL):
            hd0T[:, ec * BL + b] = hd0[ec * 128:(ec + 1) * 128]
    sh["hd0T"] = hd0T.astype(bf16)
    sh["cd0x2"] = np.broadcast_to(2.0 * cd0, (BL, HD)).astype(f32).copy()

    x = np.asarray(inputs["x"], f32)

    def core_inputs(core):
        xc = x[core * BL:(core + 1) * BL, :nt, :]      # [BL, nt, F]
        xT = np.ascontiguousarray(xc.transpose(2, 1, 0).reshape(128, nt * BL))
        m = dict(sh)
        m["xT"] = xT.astype(bf16)
        return m

    return core_inputs


def decode_out(raw, nt=T):
    """[128, nt*16] bf16 -> [BL, nt, HD] f32."""
    arr = np.asarray(raw).astype(np.float32).reshape(128, nt, 4, BL)
    return np.ascontiguousarray(arr.transpose(3, 1, 2, 0)).reshape(BL, nt, HD)


_cache = {}


def kernel(**inputs):
    nt = np.asarray(inputs["x"]).shape[1]
    if nt not in _cache:
        _cache[nt] = build_nc(nt)
    nc = _cache[nt]
    core_inputs = prepare_inputs(inputs, nt)
    in_maps = [core_inputs(c) for c in range(NCORES)]
    res = run_bass_kernel_spmd(nc, in_maps, core_ids=list(range(NCORES)))
    outs = [decode_out(res.results[c]["out"], nt) for c in range(NCORES)]
    return np.concatenate(outs, axis=0).astype(np.float32)


# revision 3
# speedup vs baseline: 1.0015x; 1.0015x over previous
"""Bahdanau encoder-decoder LSTM on 8 Trainium2 NeuronCores — v2.

Strategy: data-parallel over batch (B=32 -> 4 rows per core), weights
replicated, zero collectives. v2 restructures the per-step program to
shorten the serial cross-engine chain:

- One fused gate activation per LSTM step: the g-gate weight/bias rows
  are pre-scaled by 2 on the host so tanh(0.5*z) yields all four gate
  nonlinearities in a single ACT instruction (sigmoid via tanh).
- Cell state kept doubled (C = 2c); the update needs only 3 fused
  scalar_tensor_tensor ops (split across DVE and the Pool engine) and
  tanh(c) = tanh(0.5*C) folds the halving into the ACT scale.
- h^T produced directly by two tiny scale-matmuls (lhsT = gate slice,
  rhs = 0.5*I / I) plus one fused stt, writing straight into the
  persistent transposed h history (houtT). No per-step output DMA; one
  bulk DMA of the bf16 history at the end.
- dscore^T computed directly in transposed form with stationary
  Wdec^T tiles (no PSUM->SBUF->transpose round trip).
- Elementwise copies/scatters moved to the otherwise-idle Pool engine.
"""
import numpy as np
import ml_dtypes

import concourse.bass as bass
import concourse.tile as tile_mod
from concourse import mybir
from concourse.bass_utils import run_bass_kernel_spmd
from concourse.tile import TileContext
from concourse.vector_clock import ScopedClock

F32 = mybir.dt.float32
BF16 = mybir.dt.bfloat16
FP8 = mybir.dt.float8e4
DR = mybir.MatmulPerfMode.DoubleRow
AF = mybir.ActivationFunctionType
OP = mybir.AluOpType
bf16 = ml_dtypes.bfloat16
f8 = getattr(ml_dtypes, "float8_e4m3fn", None) or ml_dtypes.float8_e4m3

F, HE, HD, A = 128, 512, 512, 256
B, T = 32, 512
NCORES = 8
BL = B // NCORES  # 4 batch rows per core
G = 4 * HD        # 2048 gate width

# ----------------------------------------------------------------------
# Toolchain workarounds: this walrus build refuses any TPB instruction
# carrying more than one semaphore wait. Hoist extras onto standalone
# EventSemaphore instructions (engine program order keeps semantics).
_TPB_ENGINES = None


def _tpb_engines():
    global _TPB_ENGINES
    if _TPB_ENGINES is None:
        _TPB_ENGINES = {
            mybir.EngineType.PE,
            mybir.EngineType.DVE,
            mybir.EngineType.Activation,
            mybir.EngineType.Pool,
            mybir.EngineType.SP,
        }
    return _TPB_ENGINES


def split_multi_waits(nc, cap=1):
    fn = nc.m.functions[0]
    engines = _tpb_engines()
    for bb in fn.blocks:
        insts = bb.instructions
        out = []
        changed = False
        for inst in insts:
            si = inst.sync_info
            waits = None if si is None else si.on_wait
            if waits is not None and len(waits) > cap and inst.engine in engines:
                extra = list(waits[cap:])
                si.on_wait = list(waits[:cap])
                for j, w in enumerate(extra):
                    ev = mybir.InstEventSemaphore(
                        name=f"{inst.name}-xw{j}", ins=[], outs=[]
                    )
                    ev.engine = inst.engine
                    ev.sync_info = mybir.SyncInfo(on_wait=[w], on_update=[])
                    out.append(ev)
                changed = True
            out.append(inst)
        if changed:
            bb.instructions = out


_patched = False


def patch_tile_drain():
    """Same walrus limitation for the Tile tail drain."""
    global _patched
    if _patched:
        return
    _patched = True

    def _drain(self, tick_clock, wait_clock):
        drain_inst = self.nc.sync.drain()
        wait_clock.add_sem_waits(
            drain_inst.ins, ScopedClock({None: tick_clock.global_clock})
        )
        si = drain_inst.ins.sync_info
        if si is not None and si.on_wait is not None and len(si.on_wait) > 1:
            extra = list(si.on_wait[1:])
            si.on_wait = [si.on_wait[0]]
            for w in extra:
                n2 = self.nc.sync.nop()
                n2.ins.sync_info = mybir.SyncInfo(on_wait=[w], on_update=[])
        self.nc.all_engine_barrier()
        popped = self.nc._tile_sem_poison_stack.pop()
        assert popped is self._sem_poison
        self.nc.clear_and_free_semaphores(list(self.sems.allocated().values()))
        self.nc.all_engine_barrier()

    tile_mod.TileContext._drain_and_barrier = _drain


# ----------------------------------------------------------------------
def build_nc(nt=T, dbg=False):
    """Build the single-core Bass program (SPMD across 8 cores)."""
    patch_tile_drain()
    nc = bass.Bass("TRN2", target_bir_lowering=False, debug=False)
    dbg_outs = {}

    def tap(name, ap_or_tile, shape):
        if not dbg:
            return
        d = nc.dram_tensor("dbg_" + name, list(shape), ap_or_tile.dtype,
                           kind="ExternalOutput")
        nc.sync.dma_start(out=d.ap(), in_=ap_or_tile)
        dbg_outs[name] = d

    # ---- DRAM parameters (host-prepped; bf16 unless noted) ----
    d_xT = nc.dram_tensor("xT", [128, nt * BL], BF16, kind="ExternalInput")
    d_wihT = nc.dram_tensor("wihT", [128, G], BF16, kind="ExternalInput")
    d_whhT = nc.dram_tensor("whhT", [4, 128, G], FP8, kind="ExternalInput")
    d_encb = nc.dram_tensor("encb", [1, G], BF16, kind="ExternalInput")
    d_dhr = nc.dram_tensor("dhr", [4, 128, G], BF16, kind="ExternalInput")
    d_dcr = nc.dram_tensor("dcr", [4, 128, G], BF16, kind="ExternalInput")
    d_decb = nc.dram_tensor("decb", [1, G], BF16, kind="ExternalInput")
    d_wdecT = nc.dram_tensor("wdecT", [8, 128, 128], BF16, kind="ExternalInput")
    d_wencT = nc.dram_tensor("wencT", [4, 128, A], BF16, kind="ExternalInput")
    d_epb = nc.dram_tensor("epb", [2, 128, 1], F32, kind="ExternalInput")
    d_vblk = nc.dram_tensor("vblk", [128, 8 * 16], FP8, kind="ExternalInput")
    d_ones = nc.dram_tensor("ones", [1, BL], BF16, kind="ExternalInput")
    d_id4b = nc.dram_tensor("id4b", [BL, BL], BF16, kind="ExternalInput")
    d_id4f = nc.dram_tensor("id4f", [BL, BL], F32, kind="ExternalInput")
    d_half4f = nc.dram_tensor("half4f", [BL, BL], F32, kind="ExternalInput")
    d_id128 = nc.dram_tensor("id128", [128, 128], BF16, kind="ExternalInput")
    d_hd0T = nc.dram_tensor("hd0T", [128, 4 * BL], BF16, kind="ExternalInput")
    d_cd0x2 = nc.dram_tensor("cd0x2", [BL, HD], F32, kind="ExternalInput")
    d_out = nc.dram_tensor("out", [128, nt * 4 * BL], BF16, kind="ExternalOutput")

    from contextlib import ExitStack
    with TileContext(nc) as tc, ExitStack() as ctx:
        const = ctx.enter_context(tc.tile_pool(name="const", bufs=1))
        state = ctx.enter_context(tc.tile_pool(name="state", bufs=1))
        work = ctx.enter_context(tc.tile_pool(name="work", bufs=1))

        # ---- load constants into SBUF ----
        xT = const.tile([128, nt * BL], BF16)
        nc.sync.dma_start(out=xT, in_=d_xT.ap())
        wihT = const.tile([128, G], BF16)
        nc.sync.dma_start(out=wihT, in_=d_wihT.ap())
        whhT = const.tile([128, 4, G], FP8)
        for k in range(4):
            nc.sync.dma_start(out=whhT[:, k, :], in_=d_whhT.ap()[k])
        encb = const.tile([1, G], BF16)
        nc.sync.dma_start(out=encb, in_=d_encb.ap())
        dhr = const.tile([128, 4, G], BF16)
        for k in range(4):
            nc.sync.dma_start(out=dhr[:, k, :], in_=d_dhr.ap()[k])
        dcr = const.tile([128, 4, G], BF16)
        for k in range(4):
            nc.sync.dma_start(out=dcr[:, k, :], in_=d_dcr.ap()[k])
        decb = const.tile([1, G], BF16)
        nc.sync.dma_start(out=decb, in_=d_decb.ap())
        wdecT = const.tile([128, 8, 128], BF16)
        for k in range(8):
            nc.sync.dma_start(out=wdecT[:, k, :], in_=d_wdecT.ap()[k])
        wencT = const.tile([128, 4, A], BF16)
        for k in range(4):
            nc.sync.dma_start(out=wencT[:, k, :], in_=d_wencT.ap()[k])
        epb = const.tile([128, 2], F32)
        for k in range(2):
            nc.sync.dma_start(out=epb[:, k:k + 1], in_=d_epb.ap()[k])
        vblk = const.tile([128, 8 * 16], FP8)
        nc.sync.dma_start(out=vblk, in_=d_vblk.ap())
        ones = const.tile([1, BL], BF16)
        nc.sync.dma_start(out=ones, in_=d_ones.ap())
        id4b = const.tile([BL, BL], BF16)
        nc.sync.dma_start(out=id4b, in_=d_id4b.ap())
        id4f = const.tile([BL, BL], F32)
        nc.sync.dma_start(out=id4f, in_=d_id4f.ap())
        half4f = const.tile([BL, BL], F32)
        nc.sync.dma_start(out=half4f, in_=d_half4f.ap())
        id128 = const.tile([128, 128], BF16)
        nc.sync.dma_start(out=id128, in_=d_id128.ap())

        # ---- persistent state ----
        # encoder h history, transposed: [e', (ec, t, b)]
        encT = state.tile([128, 4, nt * BL], BF16)
        # fp8 shadow of encT for the DoubleRow Whh matmuls
        encT8 = state.tile([128, 4, nt * BL], FP8)
        # decoder h history, transposed: [e', (t_slot, ec, b)]; slot 0 = init
        houtT = state.tile([128, (nt + 1) * 4 * BL], BF16)
        nc.sync.dma_start(out=houtT[:, 0:4 * BL], in_=d_hd0T.ap())
        C_enc = state.tile([BL, HE], F32)   # = 2*c
        nc.vector.memset(C_enc, 0.0)
        C_dec = state.tile([BL, HD], F32)   # = 2*c
        nc.sync.dma_start(out=C_dec, in_=d_cd0x2.ap())
        enc_projT = state.tile([128, 2, BL, nt], F32)   # [a', (ac, b, t)]
        enc_tb = state.tile([128, BL, 4, HE], FP8)      # [t', (b, tc, e)]
        align = state.tile([128, 2, BL, nt], FP8)       # [a', (ac, b, t)]
        # 16 fp8 block-diag chunks of width 16 (DoubleRow k-pair layout);
        # block (b, tc) holds w^T col b at flat col 65*b + 16*tc.
        wTblk = state.tile([128, 16 * 16 + 16], FP8)
        nc.vector.memset(wTblk, 0.0)
        ctxT = state.tile([128, 4 * BL], BF16)

        # ---- per-step work tiles ----
        Gt = work.tile([BL, G], F32)
        u_t = work.tile([BL, HE], F32)
        v_t = work.tile([BL, HE], F32)
        th = work.tile([BL, HE], F32)
        oT_s = work.tile([128, 4 * BL], F32)
        dsT_sb = work.tile([128, 2 * BL], F32)
        w_u = work.tile([BL, T], F32)
        denom = work.tile([BL, 1], F32)
        rcp = work.tile([BL, 1], F32)
        diag_s = work.tile([BL, BL], F32)
        ctx_s = work.tile([BL, HE], BF16)

        def lstm_tail(z_ps, C_st, psm, toT_sl, thT_sl, out_ap):
            """Fused gate tail. z -> gates -> C update -> h^T into out_ap.

            toT_sl/thT_sl: [128, 4*BL] column ranges in the small psum tile.
            out_ap: [128, 4*BL]-shaped AP (possibly strided) for h^T.
            """
            # gates via tanh(0.5*z) (g-gate weights pre-doubled); i|f|g
            # first so the DVE chain starts sooner, o hides under it
            nc.scalar.activation(out=Gt[:, 0:3 * HE], in_=z_ps[:, 0:3 * HE],
                                 func=AF.Tanh, bias=0.0, scale=0.5)
            nc.scalar.activation(out=Gt[:, 3 * HE:], in_=z_ps[:, 3 * HE:],
                                 func=AF.Tanh, bias=0.0, scale=0.5)
            # u = (ti+1)*tg = 2*sig(i)*tanh(g)      (DVE)
            nc.vector.scalar_tensor_tensor(
                out=u_t, in0=Gt[:, 0:HE], scalar=1.0, in1=Gt[:, 2 * HE:3 * HE],
                op0=OP.add, op1=OP.mult)
            # v = (tf+1)*C = 4*sig(f)*c             (DVE)
            nc.vector.scalar_tensor_tensor(
                out=v_t, in0=Gt[:, HE:2 * HE], scalar=1.0, in1=C_st,
                op0=OP.add, op1=OP.mult)
            # C' = 0.5*v + u = 2*c_new              (DVE)
            nc.vector.scalar_tensor_tensor(
                out=C_st, in0=v_t, scalar=0.5, in1=u_t,
                op0=OP.mult, op1=OP.add)
            # th = tanh(c_new)
            nc.scalar.activation(out=th, in_=C_st, func=AF.Tanh,
                                 bias=0.0, scale=0.5)
            # toT (scaled 0.5) and thT via tiny matmuls; all to-matmuls
            # first so they run on PE while ACT computes th
            for ec in range(4):
                nc.tensor.matmul(
                    psm[:, toT_sl + ec * BL: toT_sl + (ec + 1) * BL],
                    Gt[:, 3 * HE + ec * 128: 3 * HE + (ec + 1) * 128],
                    half4f, start=True, stop=True)
            for ec in range(4):
                nc.tensor.matmul(
                    psm[:, thT_sl + ec * BL: thT_sl + (ec + 1) * BL],
                    th[:, ec * 128:(ec + 1) * 128],
                    id4f, start=True, stop=True)
            # sig(o)^T = 0.5*toT + 0.5 -> SBUF (only one PSUM input allowed
            # per DVE op, so stage this part; it is off the critical path)
            nc.vector.tensor_scalar(
                out=oT_s, in0=psm[:, toT_sl: toT_sl + 4 * BL],
                scalar1=0.5, scalar2=None, op0=OP.add)
            # h^T = sig(o)^T * thT
            nc.vector.tensor_tensor(
                out=out_ap, in0=oT_s,
                in1=psm[:, thT_sl: thT_sl + 4 * BL], op=OP.mult)

        # ================= ENCODER =================
        with tc.tile_pool(name="eps", bufs=1, space="PSUM") as eps, \
             tc.tile_pool(name="esm", bufs=1, space="PSUM") as esm:
            psm_e = esm.tile([128, 8 * BL], F32)
            for t in range(nt):
                z_ps = eps.tile([BL, G], F32, tag="z")
                for ng in range(4):
                    sl = slice(ng * HE, (ng + 1) * HE)
                    nc.tensor.matmul(z_ps[:, sl], xT[:, BL * t:BL * (t + 1)],
                                     wihT[:, sl], start=True, stop=False)
                    nc.tensor.matmul(z_ps[:, sl], ones, encb[:, sl],
                                     start=False, stop=(t == 0))
                    if t > 0:
                        for kcp in range(2):
                            nc.tensor.matmul(
                                z_ps[:, sl],
                                encT8[:, 2 * kcp:2 * kcp + 2,
                                      BL * (t - 1): BL * t],
                                whhT[:, 2 * kcp:2 * kcp + 2, sl],
                                start=False, stop=(kcp == 1), perf_mode=DR)
                lstm_tail(z_ps, C_enc, psm_e, 0, 4 * BL,
                          encT[:, :, BL * t: BL * (t + 1)])
                # fp8 shadow for the next step's DoubleRow matmuls (DVE:
                # same queue as the h^T write, no extra cross-engine hop)
                nc.vector.tensor_copy(encT8[:, :, BL * t: BL * (t + 1)],
                                      encT[:, :, BL * t: BL * (t + 1)])
                if t == 0 and dbg:
                    zc = work.tile([BL, G], F32, tag="dbg_z")
                    nc.vector.tensor_copy(zc, z_ps)
                    tap("enc_z0", zc, [BL, G])
            tap("encT", encT, [128, 4 * nt * BL])

        # ================= PHASE 2: enc_projT and enc_tb =================
        with tc.tile_pool(name="p2a", bufs=4, space="PSUM") as p2a, \
             tc.tile_pool(name="p2b", bufs=4, space="PSUM") as p2b:
            for ac in range(2):
                for b in range(BL):
                    pp = p2a.tile([128, nt], F32, tag="pp")
                    for ec in range(4):
                        rhs = encT[:, ec, b: BL * (nt - 1) + b + 1: BL]
                        nc.tensor.matmul(
                            pp, wencT[:, ec, ac * 128:(ac + 1) * 128],
                            rhs, start=(ec == 0), stop=(ec == 3))
                    nc.scalar.activation(
                        out=enc_projT[:, ac, b, :], in_=pp,
                        func=AF.Identity, bias=epb[:, ac:ac + 1], scale=1.0)
            for b in range(BL):
                for tc_i in range(nt // 128):
                    for ec in range(4):
                        src = encT[:, ec, tc_i * 512 + b: tc_i * 512 + b
                                   + BL * 127 + 1: BL]
                        pt2 = p2b.tile([128, 128], BF16, tag="pt2")
                        nc.tensor.transpose(pt2, src, id128)
                        nc.vector.tensor_copy(
                            enc_tb[:, b, tc_i, ec * 128:(ec + 1) * 128], pt2)

        tap("enc_projT", enc_projT, [128, 2 * nt * BL])
        tap("enc_tb", enc_tb, [128, 16 * HE])

        # ================= DECODER =================
        with tc.tile_pool(name="dz", bufs=1, space="PSUM") as dz, \
             tc.tile_pool(name="datt", bufs=1, space="PSUM") as datt, \
             tc.tile_pool(name="dsm", bufs=1, space="PSUM") as dsm:
            # small psum: [0:8]=dsT, [8:24]=toT, [24:40]=thT,
            # [40:56]=wT, [56:72]=ctxT
            psm = dsm.tile([128, 72], F32)
            for t in range(nt):
                hsl = houtT[:, t * 4 * BL: (t + 1) * 4 * BL]
                # bias matmuls first: independent of h_t, so the PE can run
                # them during the previous step's tail
                z_ps = dz.tile([BL, G], F32, tag="zd")
                for ng in range(4):
                    sl = slice(ng * HD, (ng + 1) * HD)
                    nc.tensor.matmul(z_ps[:, sl], ones, decb[:, sl],
                                     start=True, stop=False)
                # --- dsT: stationary Wdec^T tiles (chain head) ---
                for ac in range(2):
                    for kc in range(4):
                        nc.tensor.matmul(
                            psm[:, ac * BL:(ac + 1) * BL],
                            wdecT[:, kc * 2 + ac, :],
                            hsl[:, kc * BL:(kc + 1) * BL],
                            start=(kc == 0), stop=(kc == 3))
                # PSUM->SBUF bias staging on ACT itself: same queue as the
                # align activations, so no extra cross-engine hop
                nc.scalar.copy(dsT_sb, psm[:, 0:2 * BL])
                # --- z h-pass (fills the PE while ACT runs align) ---
                for ng in range(4):
                    sl = slice(ng * HD, (ng + 1) * HD)
                    for kc in range(4):
                        nc.tensor.matmul(
                            z_ps[:, sl], hsl[:, kc * BL:(kc + 1) * BL],
                            dhr[:, kc, sl], start=False, stop=False)
                # --- attention tanh ---
                for ac in range(2):
                    for b in range(BL):
                        nc.scalar.activation(
                            out=align[:, ac, b, :],
                            in_=enc_projT[:, ac, b, :],
                            func=AF.Tanh,
                            bias=dsT_sb[:, ac * BL + b: ac * BL + b + 1],
                            scale=1.0)
                # --- V reduction (block-diag, DoubleRow over ac pairs) ---
                lg_ps = datt.tile([BL, T], F32, tag="lg")
                for b in range(BL):
                    lhsT = vblk[:, b * 32: b * 32 + 32].rearrange(
                        "p (a w) -> p a w", a=2)[:, :, 0:BL]
                    nc.tensor.matmul(
                        lg_ps[:, :nt], lhsT, align[:, 0:2, b, :],
                        start=(b == 0), stop=(b == 3), perf_mode=DR)
                # --- softmax (V_b cancels; logits bounded) ---
                nc.scalar.activation(out=w_u[:, :nt], in_=lg_ps[:, :nt],
                                     func=AF.Exp, bias=0.0, scale=1.0,
                                     accum_out=denom)
                nc.vector.reciprocal(rcp, denom)
                # diag(1/denom): folds softmax normalization into the wT
                # transpose-matmuls, removing the ctx normalize from the chain
                nc.vector.tensor_scalar(out=diag_s, in0=id4f, scalar1=rcp,
                                        scalar2=None, op0=OP.mult)
                # --- normalized wT via scale-matmuls into fp8 block-diag ---
                # block (b, tc) base = (b*4+tc)*16, col b -> flat 65*b + 16*tc
                for tc_i in range(4):
                    wtp = psm[:, 40 + tc_i * BL: 40 + (tc_i + 1) * BL]
                    nc.tensor.matmul(wtp, w_u[:, tc_i * 128:(tc_i + 1) * 128],
                                     diag_s, start=True, stop=True)
                    dst = wTblk[:, 16 * tc_i: 16 * tc_i + 65 * (BL - 1) + 1: 65]
                    nc.vector.tensor_copy(dst, wtp)
                # --- ctx (unnormalized, DoubleRow over tc pairs) ---
                ctx_ps = datt.tile([BL, HE], F32, tag="ctx")
                q = 0
                for b in range(BL):
                    for tcp in range(2):
                        lhsT = wTblk[:, (b * 4 + 2 * tcp) * 16:
                                     (b * 4 + 2 * tcp) * 16 + 32].rearrange(
                            "p (a w) -> p a w", a=2)[:, :, 0:BL]
                        nc.tensor.matmul(
                            ctx_ps, lhsT,
                            enc_tb[:, b, 2 * tcp:2 * tcp + 2, :],
                            start=(q == 0), stop=(q == 7), perf_mode=DR)
                        q += 1
                # --- ctx is already normalized; plain cast to bf16 ---
                nc.vector.tensor_copy(ctx_s, ctx_ps)
                # --- ctxT via scale-matmuls ---
                for ec in range(4):
                    ctp = psm[:, 56 + ec * BL: 56 + (ec + 1) * BL]
                    nc.tensor.matmul(ctp, ctx_s[:, ec * 128:(ec + 1) * 128],
                                     id4b, start=True, stop=True)
                    nc.vector.tensor_copy(ctxT[:, ec * BL:(ec + 1) * BL], ctp)
                # --- cell ctx-pass (accumulate into z) ---
                for ng in range(4):
                    sl = slice(ng * HD, (ng + 1) * HD)
                    for kc in range(4):
                        nc.tensor.matmul(
                            z_ps[:, sl], ctxT[:, kc * BL:(kc + 1) * BL],
                            dcr[:, kc, sl], start=False, stop=(kc == 3))
                # --- gates + state + h^T into houtT slot t+1 ---
                lstm_tail(z_ps, C_dec, psm, 8, 24,
                          houtT[:, (t + 1) * 4 * BL: (t + 2) * 4 * BL])
                if t == 0 and dbg:
                    tap("d_dsT0", dsT_sb, [128, 2 * BL])
                    tap("d_wu0", w_u, [BL, T])
                    tap("d_denom0", denom, [BL, 1])
                    tap("d_ctxs0", ctx_s, [BL, HE])
        # one bulk DMA of the h history (excluding the init slot)
        nc.sync.dma_start(out=d_out.ap(), in_=houtT[:, 4 * BL:])

    split_multi_waits(nc)
    if dbg:
        nc._dbg_outs = dbg_outs
    return nc


# ----------------------------------------------------------------------
def _sig(x):
    return 1.0 / (1.0 + np.exp(-x))


def prepare_inputs(inputs, nt=T):
    """Host-side weight/layout prep. Returns per-core input map fn."""
    f32 = np.float32
    enc_Wih = np.asarray(inputs["enc_Wih"], f32)
    enc_Whh = np.asarray(inputs["enc_Whh"], f32)
    enc_bias = np.asarray(inputs["enc_bih"], f32) + np.asarray(inputs["enc_bhh"], f32)
    Wenc_w = np.asarray(inputs["Wenc_w"], f32)
    Wenc_b = np.asarray(inputs["Wenc_b"], f32)
    Wdec_w = np.asarray(inputs["Wdec_w"], f32)
    Wdec_b = np.asarray(inputs["Wdec_b"], f32)
    V_w = np.asarray(inputs["V_w"], f32)
    attn_bias = np.asarray(inputs["attn_bias"], f32)
    dec_Wih = np.asarray(inputs["dec_Wih"], f32)
    dec_Whh = np.asarray(inputs["dec_Whh"], f32)
    dec_bias = np.asarray(inputs["dec_bih"], f32) + np.asarray(inputs["dec_bhh"], f32)

    def g2(w):
        """Double the g-gate block (rows 2H:3H of the 4H gate dim)."""
        w = w.copy()
        hq = w.shape[0] // 4
        w[2 * hq:3 * hq] *= 2.0
        return w

    sh = {}
    sh["wihT"] = np.ascontiguousarray(g2(enc_Wih).T).astype(bf16)
    # Whh.T is [HE, 4HE]; k-chunks split HE  (fp8 for DoubleRow)
    sh["whhT"] = np.ascontiguousarray(
        g2(enc_Whh).T.reshape(4, 128, 4 * HE)).astype(f8)
    sh["encb"] = g2(enc_bias.reshape(-1, 1))[:, 0].reshape(1, -1).astype(bf16)
    dec_h_w = g2(dec_Wih[:, HD:] + dec_Whh)          # [2048, 512]
    sh["dhr"] = np.ascontiguousarray(
        dec_h_w.T.reshape(4, 128, G)).astype(bf16)
    sh["dcr"] = np.ascontiguousarray(
        g2(dec_Wih[:, :HD]).T.reshape(4, 128, G)).astype(bf16)
    sh["decb"] = g2(dec_bias.reshape(-1, 1))[:, 0].reshape(1, -1).astype(bf16)
    # Wdec^T tiles [kc, ac] -> [8, 128, 128], index kc*2+ac
    wdT = Wdec_w.T  # [HD, A]
    wt = np.zeros((8, 128, 128), f32)
    for kc in range(4):
        for ac in range(2):
            wt[kc * 2 + ac] = wdT[kc * 128:(kc + 1) * 128,
                                  ac * 128:(ac + 1) * 128]
    sh["wdecT"] = wt.astype(bf16)
    sh["wencT"] = np.ascontiguousarray(
        Wenc_w.T.reshape(4, 128, A)).astype(bf16)
    sh["epb"] = np.ascontiguousarray(
        (Wenc_b + attn_bias + Wdec_b).reshape(2, 128, 1)).astype(f32)
    # V block-diag for DoubleRow: block (b, ac) at (b*2+ac)*16, col b
    vb = np.zeros((128, 8 * 16), f32)
    for b in range(BL):
        for ac in range(2):
            vb[:, (b * 2 + ac) * 16 + b] = V_w[0, ac * 128:(ac + 1) * 128]
    sh["vblk"] = vb.astype(f8)
    sh["ones"] = np.ones((1, BL), f32).astype(bf16)
    sh["id4b"] = np.eye(BL, dtype=f32).astype(bf16)
    sh["id4f"] = np.eye(BL, dtype=f32)
    sh["half4f"] = (0.5 * np.eye(BL)).astype(f32)
    sh["id128"] = np.eye(128, dtype=f32).astype(bf16)
    # decoder init state (z0 from biases only; original biases, no g2)
    i0, f0, g0, o0 = np.split(dec_bias, 4)
    cd0 = _sig(i0) * np.tanh(g0)
    hd0 = _sig(o0) * np.tanh(cd0)
    hd0T = np.zeros((128, 4 * BL), f32)
    for ec in range(4):
        for b in range(BL):
            hd0T[:, ec * BL + b] = hd0[ec * 128:(ec + 1) * 128]
    sh["hd0T"] = hd0T.astype(bf16)
    sh["cd0x2"] = np.broadcast_to(2.0 * cd0, (BL, HD)).astype(f32).copy()

    x = np.asarray(inputs["x"], f32)

    def core_inputs(core):
        xc = x[core * BL:(core + 1) * BL, :nt, :]      # [BL, nt, F]
        xT = np.ascontiguousarray(xc.transpose(2, 1, 0).reshape(128, nt * BL))
        m = dict(sh)
        m["xT"] = xT.astype(bf16)
        return m

    return core_inputs


def decode_out(raw, nt=T):
    """[128, nt*16] bf16 -> [BL, nt, HD] f32."""
    arr = np.asarray(raw).astype(np.float32).reshape(128, nt, 4, BL)
    return np.ascontiguousarray(arr.transpose(3, 1, 2, 0)).reshape(BL, nt, HD)


_cache = {}


def kernel(**inputs):
    nt = np.asarray(inputs["x"]).shape[1]
    if nt not in _cache:
        _cache[nt] = build_nc(nt)
    nc = _cache[nt]
    core_inputs = prepare_inputs(inputs, nt)
    in_maps = [core_inputs(c) for c in range(NCORES)]
    res = run_bass_kernel_spmd(nc, in_maps, core_ids=list(range(NCORES)))
    outs = [decode_out(res.results[c]["out"], nt) for c in range(NCORES)]
    return np.concatenate(outs, axis=0).astype(np.float32)
